# revision 1
# baseline (speedup 1.0000x reference)
"""3-layer GCN encoder on 8 TRN2 NeuronCores (Bass/Tile).

Strategy (see spec sharding_hint): nodes degree-rank-dealt across 8 cores
(12500 + 44 pad rows each). Per layer, per core: shard matmul g^T = W^T x'^T
(PE, K=64), PE-transpose to node-major, 2-part AllGather into a full fp32
gather table, bulk dma_gather over two int16 windows (signed offsets from
shifted bases), padded-CSR degree-sorted blocks, strided DVE segment reduces,
alignment re-gather + add, relu/scale postproc (dis scaling commutes with
relu when biases are zero). All heavy arithmetic on device; host does index
preprocessing / sharding only.
"""

import sys

sys.path.insert(0, "/opt/trn_rl_repo")

import numpy as np

from concourse import bass, bacc, mybir, tile
import concourse.bass_utils as bass_utils
from concourse.masks import make_identity

# ---------------- problem constants (hardcoded per harness contract) -------
N = 100000
E = 1600000
F = 64  # feature width used everywhere (W3 zero-padded 32->64)
OC = 32
NCORES = 8
KB = 98  # blocks per core
SH = KB * 128  # 12544 rows per core shard
NPC = 12500  # real nodes per core

# table layout: [AG1 out: 0..50175 | zeros: 50176..50303 | AG2 out: 50304..100479 | zeros: 100480..100607]
HALF = 64 * KB  # 6272 rows per shard half
P1_SIZE = NCORES * HALF  # 50176
Z1_ROW = P1_SIZE  # 50176 (static zero tile rows 50176..50303)
P2_OFF = P1_SIZE + 128  # 50304
Z2_ROW = P2_OFF + NCORES * HALF  # 100480
T_ROWS = Z2_ROW + 128  # 100608
BASE_1 = 32768  # window1 reach [0, 65535] covers part1+Z1
BASE_2 = 83072  # window2 reach [50304, 115839] covers part2+Z2

GMAX_COLS = 52  # max slot-columns per gather group

f32 = mybir.dt.float32
i16 = mybir.dt.int16

_CACHE = {}


# ============================ host preprocessing ===========================

def _wrap_idx(vals: np.ndarray) -> np.ndarray:
    """[n] int -> [128, n//16] int16 (wrapped in 16 partitions, replicated x8)."""
    n = len(vals)
    assert n % 16 == 0
    a = vals.reshape(n // 16, 16).T.astype(np.int16)
    return np.tile(a, (8, 1))


def _preprocess(edge_index: np.ndarray):
    src = np.asarray(edge_index[0], dtype=np.int64)
    dst = np.asarray(edge_index[1], dtype=np.int64)
    deg = np.bincount(dst, minlength=N).astype(np.int64) + 1

    # global degree-rank deal: rank r -> core r%8, in-core rank j=r//8
    order = np.argsort(-deg, kind="stable")  # rank -> node
    node_core = np.empty(N, np.int32)
    node_j = np.empty(N, np.int32)
    ranks = np.arange(N)
    node_core[order] = (ranks % NCORES).astype(np.int32)
    node_j[order] = (ranks // NCORES).astype(np.int32)

    # in-core coords: j -> (k=j//128, p=j%128); table row:
    #   p<64:  c*HALF + p*98 + k        (part 1)
    #   p>=64: P2_OFF + c*HALF + (p-64)*98 + k   (part 2)
    k_of = node_j // 128
    p_of = node_j % 128
    trow = np.where(
        p_of < 64,
        node_core * HALF + p_of * KB + k_of,
        P2_OFF + node_core * HALF + (p_of - 64) * KB + k_of,
    ).astype(np.int64)

    # incoming edge lists grouped by dst
    eorder = np.argsort(dst, kind="stable")
    src_sorted = src[eorder]
    counts = np.bincount(dst, minlength=N)
    starts = np.zeros(N + 1, np.int64)
    np.cumsum(counts, out=starts[1:])

    # per node: incoming srcs' table rows + self row, split by window
    all_rows = trow[src_sorted]
    w_of_row = (all_rows >= P1_SIZE).astype(np.int8)  # 0 = window1, 1 = window2
    self_w = (trow >= P1_SIZE).astype(np.int8)

    # per-(node, window) counts
    cnt_w = np.zeros((N, 2), np.int32)
    np.add.at(cnt_w, (dst[eorder], w_of_row.astype(np.int64)), 1)
    cnt_w[np.arange(N), self_w.astype(np.int64)] += 1

    cores = []
    # shared block schedule D[k][w]: max over cores of sorted per-window counts
    percore = []
    for c in range(NCORES):
        nodes_c = order[c::NCORES]  # in-core rank j -> node
        percore.append(nodes_c)
    D = np.zeros((2, KB), np.int32)
    pi_w = []  # [w][c] -> permutation over in-core slots (j indices incl pads)
    for w in range(2):
        perms = []
        for c in range(NCORES):
            nodes_c = percore[c]
            cw = np.zeros(SH, np.int32)
            cw[: len(nodes_c)] = cnt_w[nodes_c, w]
            perm = np.argsort(-cw, kind="stable")  # slot j' -> in-core rank j
            perms.append(perm)
            sorted_c = cw[perm]
            blk_max = sorted_c.reshape(KB, 128)[:, 0]
            D[w] = np.maximum(D[w], blk_max)
        pi_w.append(perms)
    D = np.maximum(D, 1)

    # group packing: blocks -> groups with <= GMAX_COLS slot columns
    groups = [[], []]  # [w] -> list of list-of-block-ids
    for w in range(2):
        cur, cur_cols = [], 0
        for kb in range(KB):
            d = int(D[w][kb])
            if cur and cur_cols + d > GMAX_COLS:
                groups[w].append(cur)
                cur, cur_cols = [], 0
            cur.append(kb)
            cur_cols += d
        if cur:
            groups[w].append(cur)

    zrel = (Z1_ROW - BASE_1, Z2_ROW - BASE_2)
    base_w = (BASE_1, BASE_2)

    # build per-core idx blobs + align idx + per-core arrays
    idx_blob = [[None] * NCORES, [None] * NCORES]
    al_idx = [[None] * NCORES, [None] * NCORES]
    deg_arr = [None] * NCORES
    for c in range(NCORES):
        nodes_c = percore[c]
        nc_nodes = len(nodes_c)
        # per (in-core rank, window) row lists
        for w in range(2):
            perm = pi_w[w][c]
            cols_parts = []
            for g in groups[w]:
                gcols = []
                for kb in g:
                    d = int(D[w][kb])
                    block = perm[kb * 128 : (kb + 1) * 128]  # 128 in-core ranks
                    # rows matrix [128, d] filled with zrel
                    m = np.full((128, d), zrel[w], np.int64)
                    for p in range(128):
                        j = block[p]
                        if j >= nc_nodes:
                            continue
                        node = nodes_c[j]
                        rows = all_rows[starts[node] : starts[node + 1]]
                        rows = rows[w_of_row[starts[node] : starts[node + 1]] == w]
                        lst = list(rows)
                        if self_w[node] == w:
                            lst.append(trow[node])
                        m[p, : len(lst)] = np.asarray(lst, np.int64) - base_w[w]
                    gcols.append(m)
                gm = np.concatenate(gcols, axis=1)  # [128, cols_g]
                gm = np.concatenate(
                    [gm, np.full((128, 1), zrel[w], np.int64)], axis=1
                )  # sentinel col
                # positions: col-major: pos = col*128 + p
                cols_parts.append(_wrap_idx(gm.T.ravel()))
            idx_blob[w][c] = np.concatenate(cols_parts, axis=1)

            # align gather: master pos (k*128+p) -> u_w dram row p'*98+k'
            inv = np.empty(SH, np.int64)  # in-core rank j -> slot index in pi_w
            inv[perm] = np.arange(SH)
            jj = np.arange(SH)  # master position index = k*128 + p
            kk = jj // 128
            pp = jj % 128
            j_of_pos = kk * 128 + pp  # in-core rank at master (p, k) = k*128+p
            sw = inv[j_of_pos]
            al = (sw % 128) * KB + sw // 128  # u_w dram row
            al_idx[w][c] = _wrap_idx(al)

        dg = np.full((128, KB), 1e30, np.float32)
        jj = np.arange(nc_nodes)
        dg[jj % 128, jj // 128] = deg[nodes_c].astype(np.float32)
        deg_arr[c] = dg

    maxg = max(
        sum(int(D[w][kb]) for kb in g) + 1 for w in range(2) for g in groups[w]
    )
    meta = dict(
        D=D,
        groups=groups,
        maxg=maxg,
        percore=percore,
        idx_blob=idx_blob,
        al_idx=al_idx,
        deg_arr=deg_arr,
    )
    return meta


# ============================ device kernel ================================

def _build(meta, with_bias: bool, reps: int = 1):
    import os

    stop = os.environ.get("KBUILD_STOP", "full")  # ag|gather|reduce|align|full
    nlayers = int(os.environ.get("KBUILD_NLAYERS", "3"))
    max_groups = int(os.environ.get("KBUILD_MAX_GROUPS", "9999"))
    skip_ag = os.environ.get("KBUILD_SKIP_AG", "") == "1"
    nwin = int(os.environ.get("KBUILD_NWIN", "2"))
    D, groups = meta["D"], meta["groups"]
    nc = bacc.Bacc("TRN2", target_bir_lowering=False, debug=False, num_devices=NCORES)

    x_in = nc.dram_tensor("x_nm", [128, KB * F], f32, kind="ExternalInput")
    deg_in = nc.dram_tensor("deg", [128, KB], f32, kind="ExternalInput")
    w_in = [
        nc.dram_tensor(f"w{l}", [F, F], f32, kind="ExternalInput") for l in (1, 2, 3)
    ]
    b_in = [
        nc.dram_tensor(f"b{l}", [1, F], f32, kind="ExternalInput") for l in (1, 2, 3)
    ]
    idx_in = [
        nc.dram_tensor(
            f"idxw{w + 1}", list(meta["idx_blob"][w][0].shape), i16, kind="ExternalInput"
        )
        for w in range(2)
    ]
    al_in = [
        nc.dram_tensor(f"alw{w + 1}", [128, SH // 16], i16, kind="ExternalInput")
        for w in range(2)
    ]
    x_out = [
        nc.dram_tensor(f"x{l}o", [128, KB * F], f32, kind="ExternalOutput")
        for l in (1, 2, 3)
    ]

    with tile.TileContext(nc) as tc:
        with (
            tc.tile_pool(name="const", bufs=1) as cpool,
            tc.tile_pool(name="sbuf", bufs=2) as sb,
            tc.tile_pool(name="big", bufs=1) as bigp,
            tc.tile_pool(name="msgs", bufs=2) as msp,
            tc.tile_pool(name="psum_mm", bufs=2, space="PSUM") as ps_mm,
            tc.tile_pool(name="psum_tr", bufs=2, space="PSUM") as ps_tr,
            tc.tile_pool(name="dram", bufs=1, space="DRAM") as dr,
        ):
            # ---- constants ----
            ident = cpool.tile([128, 128], f32)
            make_identity(nc, ident[:])
            w_sb = []
            for l in range(3):
                t = cpool.tile([F, F], f32, tag=f"w{l}")
                nc.sync.dma_start(out=t[:], in_=w_in[l][:, :])
                w_sb.append(t)
            b_sb = []
            if with_bias:
                for l in range(3):
                    t = cpool.tile([1, F], f32, tag=f"b{l}")
                    nc.sync.dma_start(out=t[:], in_=b_in[l][:, :])
                    b_sb.append(t)
            deg_sb = cpool.tile([128, KB], f32)
            nc.sync.dma_start(out=deg_sb[:], in_=deg_in[:, :])
            al_sb = []
            for w in range(2):
                t = cpool.tile([128, SH // 16], i16, tag=f"al{w}")
                nc.sync.dma_start(out=t[:], in_=al_in[w][:, :])
                al_sb.append(t)

            # dis = rsqrt(deg), dis2 = 1/deg  (Newton-refined)
            r0 = cpool.tile([128, KB], f32, tag="r0")
            nc.vector.reciprocal(out=r0[:], in_=deg_sb[:])
            tmp = cpool.tile([128, KB], f32, tag="rt")
            nc.vector.tensor_tensor(
                out=tmp[:], in0=deg_sb[:], in1=r0[:], op=mybir.AluOpType.mult
            )
            nc.vector.tensor_scalar(
                out=tmp[:], in0=tmp[:], scalar1=-1.0, scalar2=2.0,
                op0=mybir.AluOpType.mult, op1=mybir.AluOpType.add,
            )
            dis2_sb = cpool.tile([128, KB], f32, tag="dis2")
            nc.vector.tensor_tensor(
                out=dis2_sb[:], in0=r0[:], in1=tmp[:], op=mybir.AluOpType.mult
            )
            dis_sb = cpool.tile([128, KB], f32, tag="dis")
            nc.scalar.sqrt(out=dis_sb[:], in_=dis2_sb[:])
            # one Newton step for sqrt: dis = 0.5*dis*(3 - deg*dis^2)
            s2 = cpool.tile([128, KB], f32, tag="s2")
            nc.vector.tensor_tensor(
                out=s2[:], in0=dis_sb[:], in1=dis_sb[:], op=mybir.AluOpType.mult
            )
            nc.vector.tensor_tensor(
                out=s2[:], in0=s2[:], in1=deg_sb[:], op=mybir.AluOpType.mult
            )
            nc.vector.tensor_scalar(
                out=s2[:], in0=s2[:], scalar1=-0.5, scalar2=1.5,
                op0=mybir.AluOpType.mult, op1=mybir.AluOpType.add,
            )
            nc.vector.tensor_tensor(
                out=dis_sb[:], in0=dis_sb[:], in1=s2[:], op=mybir.AluOpType.mult
            )

            # ---- DRAM scratch ----
            table = dr.tile([T_ROWS, F], f32)
            g_shard = dr.tile([SH, F], f32)
            u_dram = [
                dr.tile([SH, F], f32, tag=f"u{w}", name=f"u_dram{w}") for w in range(2)
            ]

            # static zero tiles in table (rows Z1_ROW.. and Z2_ROW..)
            ztile = cpool.tile([128, F], f32, tag="z")
            nc.vector.memset(ztile[:], 0.0)
            nc.sync.dma_start(out=table[Z1_ROW : Z1_ROW + 128, :], in_=ztile[:])
            nc.sync.dma_start(out=table[Z2_ROW : Z2_ROW + 128, :], in_=ztile[:])

            # persistent activation x'^T
            xpT = bigp.tile([F, SH], f32, tag="xpT")

            def transpose_to_xpT(xp_sb):
                """xp_sb [128, KB*F] node-major -> xpT [F, SH] feature-major."""
                for k in range(KB):
                    pt = ps_tr.tile([F, 128], f32, space="PSUM", tag="trF")
                    nc.tensor.transpose(
                        out=pt[:],
                        in_=xp_sb[:, k * F : (k + 1) * F],
                        identity=ident[:],
                    )
                    nc.scalar.copy(out=xpT[:, k * 128 : (k + 1) * 128], in_=pt[:])

            # ---- layer-0 front: x' = dis * x ----
            # (big-buffer tag sharing to fit SBUF: xnm->gT, xp->uw, outb->gnm)
            x_nm = bigp.tile([128, KB * F], f32, tag="gT")
            nc.sync.dma_start(out=x_nm[:], in_=x_in[:, :])
            xp0 = bigp.tile([128, KB * F], f32, tag="uw")
            for k in range(KB):
                nc.scalar.mul(
                    out=xp0[:, k * F : (k + 1) * F],
                    in_=x_nm[:, k * F : (k + 1) * F],
                    mul=dis_sb[:, k : k + 1],
                )
            transpose_to_xpT(xp0)

            for _rep in range(reps):
                for l in range(nlayers):
                    # ---- A: g^T = W^T @ x'^T ----
                    gT = bigp.tile([F, SH], f32, tag="gT")
                    CH = 512
                    for c0 in range(0, SH, CH):
                        cw = min(CH, SH - c0)
                        pm = ps_mm.tile([F, CH], f32, space="PSUM", tag="mm")
                        nc.tensor.matmul(
                            out=pm[:, :cw],
                            lhsT=w_sb[l][:],
                            rhs=xpT[:, c0 : c0 + cw],
                            start=True,
                            stop=True,
                        )
                        nc.scalar.copy(out=gT[:, c0 : c0 + cw], in_=pm[:, :cw])

                    # ---- B: transpose to node-major + store + AllGather ----
                    g_nm = bigp.tile([128, KB * F], f32, tag="gnm")
                    for k in range(KB):
                        pt = ps_tr.tile([128, F], f32, space="PSUM", tag="trB")
                        nc.tensor.transpose(
                            out=pt[:],
                            in_=gT[:, k * 128 : (k + 1) * 128],
                            identity=ident[:64, :64],
                        )
                        nc.scalar.copy(out=g_nm[:, k * F : (k + 1) * F], in_=pt[:])
                    # halves: partitions 0..63 -> part1 rows, 64..127 -> part2 rows
                    nc.sync.dma_start(out=g_shard[0:HALF, :], in_=g_nm[0:64, :])
                    nc.sync.dma_start(out=g_shard[HALF:SH, :], in_=g_nm[64:128, :])
                    if not skip_ag:
                        nc.gpsimd.collective_compute(
                            "AllGather",
                            mybir.AluOpType.bypass,
                            replica_groups=[list(range(NCORES))],
                            ins=[g_shard[0:HALF, :].opt()],
                            outs=[table[0:P1_SIZE, :].opt()],
                        )
                        nc.gpsimd.collective_compute(
                            "AllGather",
                            mybir.AluOpType.bypass,
                            replica_groups=[list(range(NCORES))],
                            ins=[g_shard[HALF:SH, :].opt()],
                            outs=[table[P2_OFF : P2_OFF + P1_SIZE, :].opt()],
                        )

                    if stop == "ag":
                        dbg = sb.tile([128, F], f32, tag="dbg")
                        nc.sync.dma_start(out=dbg[:], in_=table[0:128, :])
                        nc.sync.dma_start(out=x_out[l][:, 0:F], in_=dbg[:])
                        continue

                    # ---- C: window gathers + segment reduces ----
                    for w in range(nwin):
                        u_w = bigp.tile([128, KB * F], f32, tag="uw")
                        in_ap = (
                            table[BASE_1:P2_OFF, :]
                            if w == 0
                            else table[BASE_2:, :]
                        )
                        off8 = 0
                        for g in groups[w][:max_groups]:
                            gcols = sum(int(D[w][kb]) for kb in g) + 1
                            nidx = gcols * 128
                            idx_sb = sb.tile([128, nidx // 16], i16, tag="idx")
                            nc.sync.dma_start(
                                out=idx_sb[:],
                                in_=idx_in[w][:, off8 : off8 + nidx // 16],
                            )
                            msgs = msp.tile([128, meta["maxg"], F], f32, tag="msgs")
                            nc.gpsimd.dma_gather(
                                out_ap=msgs[:, :gcols, :],
                                in_ap=in_ap,
                                idxs_ap=idx_sb[:],
                                num_idxs=nidx,
                                num_idxs_reg=nidx,
                                elem_size=F,
                                single_packet=False,
                            )
                            if stop == "gather":
                                nc.scalar.copy(
                                    out=u_w[:, 0:F],
                                    in_=msgs[:, 0, :],
                                )
                            else:
                                loc = 0
                                for kb in g:
                                    d = int(D[w][kb])
                                    nc.vector.tensor_reduce(
                                        out=u_w[:, kb * F : (kb + 1) * F],
                                        in_=msgs[:, loc : loc + d, :].rearrange(
                                            "p d f -> p f d"
                                        ),
                                        axis=mybir.AxisListType.X,
                                        op=mybir.AluOpType.add,
                                    )
                                    loc += d
                            off8 += nidx // 16
                        nc.sync.dma_start(out=u_dram[w][:, :], in_=u_w[:])
                    if stop in ("gather", "reduce"):
                        dbg = sb.tile([128, F], f32, tag="dbg")
                        nc.sync.dma_start(out=dbg[:], in_=u_dram[0][0:128, :])
                        nc.sync.dma_start(out=x_out[l][:, 0:F], in_=dbg[:])
                        continue

                    # ---- D: align + add ----
                    u_al = []
                    for w in range(2):
                        t = bigp.tile([128, KB, F], f32, tag="gT" if w else "ual0")
                        nc.gpsimd.dma_gather(
                            out_ap=t[:],
                            in_ap=u_dram[w][:, :],
                            idxs_ap=al_sb[w][:],
                            num_idxs=SH,
                            num_idxs_reg=SH,
                            elem_size=F,
                            single_packet=False,
                        )
                        u_al.append(t)
                    u = u_al[0][:].rearrange("p k f -> p (k f)")
                    nc.vector.tensor_tensor(
                        out=u,
                        in0=u,
                        in1=u_al[1][:].rearrange("p k f -> p (k f)"),
                        op=mybir.AluOpType.add,
                    )
                    if stop == "align":
                        nc.sync.dma_start(out=x_out[l][:, :], in_=u)
                        continue

                    # ---- E: postproc ----
                    out_sb = bigp.tile([128, KB * F], f32, tag="gnm")
                    if l < 2:
                        xp = bigp.tile([128, KB * F], f32, tag="uw")
                    if not with_bias:
                        # r = relu(u); out = dis*r; x' = dis2*r
                        r = u_al[1][:].rearrange("p k f -> p (k f)")
                        if l < 2:
                            nc.scalar.activation(
                                out=r, in_=u, func=mybir.ActivationFunctionType.Relu
                            )
                        for k in range(KB):
                            s = slice(k * F, (k + 1) * F)
                            if l < 2:
                                nc.scalar.mul(
                                    out=out_sb[:, s], in_=r[:, s], mul=dis_sb[:, k : k + 1]
                                )
                                nc.scalar.mul(
                                    out=xp[:, s], in_=r[:, s], mul=dis2_sb[:, k : k + 1]
                                )
                            else:
                                nc.scalar.mul(
                                    out=out_sb[:, s], in_=u[:, s], mul=dis_sb[:, k : k + 1]
                                )
                    else:
                        # v = dis*u ; t = relu(v + b) (layers 1,2) / t = v + b (layer 3)
                        # out = t ; x' = dis*t
                        v = u_al[1][:].rearrange("p k f -> p (k f)")
                        for k in range(KB):
                            s = slice(k * F, (k + 1) * F)
                            nc.scalar.mul(
                                out=v[:, s], in_=u[:, s], mul=dis_sb[:, k : k + 1]
                            )
                        bb = b_sb[l][:].to_broadcast([128, F])
                        for k in range(KB):
                            s = slice(k * F, (k + 1) * F)
                            nc.vector.tensor_tensor(
                                out=out_sb[:, s], in0=v[:, s], in1=bb, op=mybir.AluOpType.add
                            )
                        if l < 2:
                            nc.scalar.activation(
                                out=out_sb[:],
                                in_=out_sb[:],
                                func=mybir.ActivationFunctionType.Relu,
                            )
                            for k in range(KB):
                                s = slice(k * F, (k + 1) * F)
                                nc.scalar.mul(
                                    out=xp[:, s],
                                    in_=out_sb[:, s],
                                    mul=dis_sb[:, k : k + 1],
                                )
                    nc.sync.dma_start(out=x_out[l][:, :], in_=out_sb[:])

                    # ---- F: next-layer x'^T ----
                    if l < 2:
                        transpose_to_xpT(xp)

    nc.compile()
    return nc


# ============================ entry point =================================

def _get_compiled(edge_index, biases_zero, reps):
    key = ("k", int(np.asarray(edge_index).sum() & 0xFFFFFFF), biases_zero, reps)
    if key not in _CACHE:
        meta = _preprocess(np.asarray(edge_index))
        nc = _build(meta, with_bias=not biases_zero, reps=reps)
        _CACHE[key] = (meta, nc)
    return _CACHE[key]


def _prepare(x, edge_index, W1, b1, W2, b2, W3, b3, _reps=1):
    x = np.asarray(x, np.float32)
    biases_zero = all(
        float(np.abs(np.asarray(b)).max()) == 0.0 for b in (b1, b2, b3)
    )
    meta, nc = _get_compiled(edge_index, biases_zero, _reps)
    percore, deg_arr = meta["percore"], meta["deg_arr"]

    W3p = np.zeros((F, F), np.float32)
    W3p[:, :OC] = np.asarray(W3, np.float32)
    b3p = np.zeros((F,), np.float32)
    b3p[:OC] = np.asarray(b3, np.float32)
    Ws = [np.asarray(W1, np.float32), np.asarray(W2, np.float32), W3p]
    bs = [
        np.asarray(b1, np.float32).reshape(1, F),
        np.asarray(b2, np.float32).reshape(1, F),
        b3p.reshape(1, F),
    ]

    in_maps = []
    for c in range(NCORES):
        nodes_c = percore[c]
        x_nm = np.zeros((128, KB * F), np.float32)
        jj = np.arange(len(nodes_c))
        kk, pp = jj // 128, jj % 128
        x_nm_3d = x_nm.reshape(128, KB, F)
        x_nm_3d[pp, kk, :] = x[nodes_c]
        m = {
            "x_nm": x_nm,
            "deg": deg_arr[c],
            "w1": Ws[0], "w2": Ws[1], "w3": Ws[2],
            "b1": bs[0], "b2": bs[1], "b3": bs[2],
            "idxw1": meta["idx_blob"][0][c],
            "idxw2": meta["idx_blob"][1][c],
            "alw1": meta["al_idx"][0][c],
            "alw2": meta["al_idx"][1][c],
        }
        in_maps.append(m)

    return meta, nc, in_maps


def kernel(x, edge_index, W1, b1, W2, b2, W3, b3, _reps=1):
    meta, nc, in_maps = _prepare(x, edge_index, W1, b1, W2, b2, W3, b3, _reps)
    percore = meta["percore"]
    res = bass_utils.run_bass_kernel_spmd(nc, in_maps, core_ids=list(range(NCORES)))

    # unshard: x_out tiles [128, KB*F] -> per-node rows
    out = np.empty((N, 160), np.float32)
    for c in range(NCORES):
        nodes_c = percore[c]
        jj = np.arange(len(nodes_c))
        kk, pp = jj // 128, jj % 128
        x1 = res.results[c]["x1o"].reshape(128, KB, F)[pp, kk, :]
        x2 = res.results[c]["x2o"].reshape(128, KB, F)[pp, kk, :]
        x3 = res.results[c]["x3o"].reshape(128, KB, F)[pp, kk, :OC]
        out[nodes_c] = np.concatenate([x3, x2, x1], axis=1)
    return out



# revision 8
# speedup vs baseline: 3.3050x; 3.3050x over previous
"""3-layer GCN encoder on 8 TRN2 NeuronCores (Bass/Tile).

Strategy (see spec sharding_hint): nodes degree-rank-dealt across 8 cores
(12500 + 44 pad rows each). Per layer, per core: shard matmul g^T = W^T x'^T
(PE, K=64), PE-transpose to node-major, 2-part AllGather into a full fp32
gather table, bulk dma_gather over two int16 windows (signed offsets from
shifted bases), padded-CSR degree-sorted blocks, strided DVE segment reduces,
alignment re-gather + add, relu/scale postproc (dis scaling commutes with
relu when biases are zero). All heavy arithmetic on device; host does index
preprocessing / sharding only.
"""

import sys

sys.path.insert(0, "/opt/trn_rl_repo")

import numpy as np

from concourse import bass, bacc, mybir, tile
import concourse.bass_utils as bass_utils
from concourse.masks import make_identity

# ---------------- problem constants (hardcoded per harness contract) -------
N = 100000
E = 1600000
F = 64  # feature width used everywhere (W3 zero-padded 32->64)
OC = 32
NCORES = 8
KB = 98  # blocks per core
SH = KB * 128  # 12544 rows per core shard
NPC = 12500  # real nodes per core

# table layout: [AG1 out: 0..50175 | zeros: 50176..50303 | AG2 out: 50304..100479 | zeros: 100480..100607]
HALF = 64 * KB  # 6272 rows per shard half
P1_SIZE = NCORES * HALF  # 50176
Z1_ROW = P1_SIZE  # 50176 (static zero tile rows 50176..50303)
P2_OFF = P1_SIZE + 128  # 50304
Z2_ROW = P2_OFF + NCORES * HALF  # 100480
T_ROWS = Z2_ROW + 128  # 100608
BASE_1 = 32768  # window1 reach [0, 65535] covers part1+Z1
BASE_2 = 83072  # window2 reach [50304, 115839] covers part2+Z2

GMAX_COLS = 20  # max slot-columns per gather group

f32 = mybir.dt.float32
i16 = mybir.dt.int16

_CACHE = {}


# ============================ host preprocessing ===========================

def _wrap_idx(vals: np.ndarray) -> np.ndarray:
    """[n] int -> [128, n//16] int16 (wrapped in 16 partitions, replicated x8)."""
    n = len(vals)
    assert n % 16 == 0
    a = vals.reshape(n // 16, 16).T.astype(np.int16)
    return np.tile(a, (8, 1))


def _preprocess(edge_index: np.ndarray):
    src = np.asarray(edge_index[0], dtype=np.int64)
    dst = np.asarray(edge_index[1], dtype=np.int64)
    deg = np.bincount(dst, minlength=N).astype(np.int64) + 1

    # global degree-rank deal: rank r -> core r%8, in-core rank j=r//8
    order = np.argsort(-deg, kind="stable")  # rank -> node
    node_core = np.empty(N, np.int32)
    node_j = np.empty(N, np.int32)
    ranks = np.arange(N)
    node_core[order] = (ranks % NCORES).astype(np.int32)
    node_j[order] = (ranks // NCORES).astype(np.int32)

    # in-core coords: j -> (k=j//128, p=j%128); table row:
    #   p<64:  c*HALF + p*98 + k        (part 1)
    #   p>=64: P2_OFF + c*HALF + (p-64)*98 + k   (part 2)
    k_of = node_j // 128
    p_of = node_j % 128
    trow = np.where(
        p_of < 64,
        node_core * HALF + p_of * KB + k_of,
        P2_OFF + node_core * HALF + (p_of - 64) * KB + k_of,
    ).astype(np.int64)

    # incoming edge lists grouped by dst
    eorder = np.argsort(dst, kind="stable")
    src_sorted = src[eorder]
    counts = np.bincount(dst, minlength=N)
    starts = np.zeros(N + 1, np.int64)
    np.cumsum(counts, out=starts[1:])

    # per node: incoming srcs' table rows + self row, split by window
    all_rows = trow[src_sorted]
    w_of_row = (all_rows >= P1_SIZE).astype(np.int8)  # 0 = window1, 1 = window2
    self_w = (trow >= P1_SIZE).astype(np.int8)

    # per-(node, window) counts (self-loop handled on-chip: u += g_nm)
    cnt_w = np.zeros((N, 2), np.int32)
    np.add.at(cnt_w, (dst[eorder], w_of_row.astype(np.int64)), 1)

    cores = []
    # shared block schedule D[k][w]: max over cores of sorted per-window counts
    percore = []
    for c in range(NCORES):
        nodes_c = order[c::NCORES]  # in-core rank j -> node
        percore.append(nodes_c)
    D = np.zeros((2, KB), np.int32)
    pi_w = []  # [w][c] -> permutation over in-core slots (j indices incl pads)
    for w in range(2):
        perms = []
        for c in range(NCORES):
            nodes_c = percore[c]
            cw = np.zeros(SH, np.int32)
            cw[: len(nodes_c)] = cnt_w[nodes_c, w]
            perm = np.argsort(-cw, kind="stable")  # slot j' -> in-core rank j
            perms.append(perm)
            sorted_c = cw[perm]
            blk_max = sorted_c.reshape(KB, 128)[:, 0]
            D[w] = np.maximum(D[w], blk_max)
        pi_w.append(perms)
    D = np.maximum(D, 1)

    # group packing: blocks -> groups with <= GMAX_COLS slot columns
    groups = [[], []]  # [w] -> list of list-of-block-ids
    for w in range(2):
        cur, cur_cols = [], 0
        for kb in range(KB):
            d = int(D[w][kb])
            if cur and cur_cols + d > GMAX_COLS:
                groups[w].append(cur)
                cur, cur_cols = [], 0
            cur.append(kb)
            cur_cols += d
        if cur:
            groups[w].append(cur)

    zrel = (Z1_ROW - BASE_1, Z2_ROW - BASE_2)
    base_w = (BASE_1, BASE_2)

    # build per-core idx blobs + align idx + per-core arrays
    idx_blob = [[None] * NCORES, [None] * NCORES]
    al_idx = [[None] * NCORES, [None] * NCORES]
    deg_arr = [None] * NCORES
    for c in range(NCORES):
        nodes_c = percore[c]
        nc_nodes = len(nodes_c)
        # per (in-core rank, window) row lists
        for w in range(2):
            perm = pi_w[w][c]
            cols_parts = []
            for g in groups[w]:
                gcols = []
                for kb in g:
                    d = int(D[w][kb])
                    block = perm[kb * 128 : (kb + 1) * 128]  # 128 in-core ranks
                    # rows matrix [128, d] filled with zrel
                    m = np.full((128, d), zrel[w], np.int64)
                    for p in range(128):
                        j = block[p]
                        if j >= nc_nodes:
                            continue
                        node = nodes_c[j]
                        rows = all_rows[starts[node] : starts[node + 1]]
                        rows = rows[w_of_row[starts[node] : starts[node + 1]] == w]
                        lst = list(rows)
                        if self_w[node] == w:
                            lst.append(trow[node])
                        m[p, : len(lst)] = np.asarray(lst, np.int64) - base_w[w]
                    gcols.append(m)
                gm = np.concatenate(gcols, axis=1)  # [128, cols_g]
                gm = np.concatenate(
                    [gm, np.full((128, 1), zrel[w], np.int64)], axis=1
                )  # sentinel col
                # positions: col-major: pos = col*128 + p
                cols_parts.append(_wrap_idx(gm.T.ravel()))
            idx_blob[w][c] = np.concatenate(cols_parts, axis=1)

            # align gather: master pos (k*128+p) -> u_w dram row p'*98+k'
            inv = np.empty(SH, np.int64)  # in-core rank j -> slot index in pi_w
            inv[perm] = np.arange(SH)
            jj = np.arange(SH)  # master position index = k*128 + p
            kk = jj // 128
            pp = jj % 128
            j_of_pos = kk * 128 + pp  # in-core rank at master (p, k) = k*128+p
            sw = inv[j_of_pos]
            al = (sw % 128) * KB + sw // 128  # u_w dram row
            al_idx[w][c] = _wrap_idx(al)

        dg = np.full((128, KB), 1e30, np.float32)
        jj = np.arange(nc_nodes)
        dg[jj % 128, jj // 128] = deg[nodes_c].astype(np.float32)
        deg_arr[c] = dg

    maxg = max(
        sum(int(D[w][kb]) for kb in g) + 1 for w in range(2) for g in groups[w]
    )
    meta = dict(
        D=D,
        groups=groups,
        maxg=maxg,
        percore=percore,
        idx_blob=idx_blob,
        al_idx=al_idx,
        deg_arr=deg_arr,
    )
    return meta


# ============================ device kernel ================================

def _build(meta, with_bias: bool, reps: int = 1):
    import os

    stop = os.environ.get("KBUILD_STOP", "full")  # ag|gather|reduce|align|full
    nlayers = int(os.environ.get("KBUILD_NLAYERS", "3"))
    max_groups = int(os.environ.get("KBUILD_MAX_GROUPS", "9999"))
    skip_ag = os.environ.get("KBUILD_SKIP_AG", "") == "1"
    nwin = int(os.environ.get("KBUILD_NWIN", "2"))
    D, groups = meta["D"], meta["groups"]
    nq = int(os.environ.get("KBUILD_NQ", "4"))
    nc = bacc.Bacc("TRN2", target_bir_lowering=False, debug=False,
                   num_devices=NCORES, num_swdge_queues=nq)

    x_in = nc.dram_tensor("x_nm", [128, KB * F], f32, kind="ExternalInput")
    deg_in = nc.dram_tensor("deg", [128, KB], f32, kind="ExternalInput")
    w_in = [
        nc.dram_tensor(f"w{l}", [F, F], f32, kind="ExternalInput") for l in (1, 2, 3)
    ]
    b_in = [
        nc.dram_tensor(f"b{l}", [1, F], f32, kind="ExternalInput") for l in (1, 2, 3)
    ]
    idx_in = [
        nc.dram_tensor(
            f"idxw{w + 1}", list(meta["idx_blob"][w][0].shape), i16, kind="ExternalInput"
        )
        for w in range(2)
    ]
    al_in = [
        nc.dram_tensor(f"alw{w + 1}", [128, SH // 16], i16, kind="ExternalInput")
        for w in range(2)
    ]
    x_out = [
        nc.dram_tensor(f"x{l}o", [128, KB * F], f32, kind="ExternalOutput")
        for l in (1, 2, 3)
    ]

    with tile.TileContext(nc) as tc:
        with (
            tc.tile_pool(name="const", bufs=1) as cpool,
            tc.tile_pool(name="sbuf", bufs=2) as sb,
            tc.tile_pool(name="big", bufs=1) as bigp,
            tc.tile_pool(name="msgs", bufs=1) as msp,
            tc.tile_pool(name="psum_mm", bufs=2, space="PSUM") as ps_mm,
            tc.tile_pool(name="psum_tr", bufs=2, space="PSUM") as ps_tr,
            tc.tile_pool(name="dram", bufs=1, space="DRAM") as dr,
        ):
            # ---- constants ----
            ident = cpool.tile([128, 128], f32)
            make_identity(nc, ident[:])
            w_sb = []
            for l in range(3):
                t = cpool.tile([F, F], f32, tag=f"w{l}")
                nc.sync.dma_start(out=t[:], in_=w_in[l][:, :])
                w_sb.append(t)
            b_sb = []
            if with_bias:
                for l in range(3):
                    t = cpool.tile([1, F], f32, tag=f"b{l}")
                    nc.sync.dma_start(out=t[:], in_=b_in[l][:, :])
                    b_sb.append(t)
            deg_sb = cpool.tile([128, KB], f32)
            nc.sync.dma_start(out=deg_sb[:], in_=deg_in[:, :])
            al_sb = []
            for w in range(2):
                t = cpool.tile([128, SH // 16], i16, tag=f"al{w}")
                nc.sync.dma_start(out=t[:], in_=al_in[w][:, :])
                al_sb.append(t)

            # dis = rsqrt(deg), dis2 = 1/deg  (Newton-refined)
            r0 = cpool.tile([128, KB], f32, tag="r0")
            nc.vector.reciprocal(out=r0[:], in_=deg_sb[:])
            tmp = cpool.tile([128, KB], f32, tag="rt")
            nc.vector.tensor_tensor(
                out=tmp[:], in0=deg_sb[:], in1=r0[:], op=mybir.AluOpType.mult
            )
            nc.vector.tensor_scalar(
                out=tmp[:], in0=tmp[:], scalar1=-1.0, scalar2=2.0,
                op0=mybir.AluOpType.mult, op1=mybir.AluOpType.add,
            )
            dis2_sb = cpool.tile([128, KB], f32, tag="dis2")
            nc.vector.tensor_tensor(
                out=dis2_sb[:], in0=r0[:], in1=tmp[:], op=mybir.AluOpType.mult
            )
            dis_sb = cpool.tile([128, KB], f32, tag="dis")
            nc.scalar.sqrt(out=dis_sb[:], in_=dis2_sb[:])
            # one Newton step for sqrt: dis = 0.5*dis*(3 - deg*dis^2)
            s2 = cpool.tile([128, KB], f32, tag="s2")
            nc.vector.tensor_tensor(
                out=s2[:], in0=dis_sb[:], in1=dis_sb[:], op=mybir.AluOpType.mult
            )
            nc.vector.tensor_tensor(
                out=s2[:], in0=s2[:], in1=deg_sb[:], op=mybir.AluOpType.mult
            )
            nc.vector.tensor_scalar(
                out=s2[:], in0=s2[:], scalar1=-0.5, scalar2=1.5,
                op0=mybir.AluOpType.mult, op1=mybir.AluOpType.add,
            )
            nc.vector.tensor_tensor(
                out=dis_sb[:], in0=dis_sb[:], in1=s2[:], op=mybir.AluOpType.mult
            )

            # ---- DRAM scratch ----
            table = dr.tile([T_ROWS, F], f32)
            g_shard = dr.tile([SH, F], f32)
            u_dram = [
                dr.tile([SH, F], f32, tag=f"u{w}", name=f"u_dram{w}") for w in range(2)
            ]

            # static zero tiles in table (rows Z1_ROW.. and Z2_ROW..)
            ztile = cpool.tile([128, F], f32, tag="z")
            nc.vector.memset(ztile[:], 0.0)
            nc.sync.dma_start(out=table[Z1_ROW : Z1_ROW + 128, :], in_=ztile[:])
            nc.sync.dma_start(out=table[Z2_ROW : Z2_ROW + 128, :], in_=ztile[:])

            # persistent activation x'^T
            xpT = bigp.tile([F, SH], f32, tag="xpT")
            qi = [0]  # round-robin SWDGE queue counter

            def next_q():
                q = qi[0] % nq
                qi[0] += 1
                return q

            def transpose_to_xpT(xp_sb):
                """xp_sb [128, KB*F] node-major -> xpT [F, SH] feature-major."""
                for k in range(KB):
                    pt = ps_tr.tile([F, 128], f32, space="PSUM", tag="trF")
                    nc.tensor.transpose(
                        out=pt[:],
                        in_=xp_sb[:, k * F : (k + 1) * F],
                        identity=ident[:],
                    )
                    nc.scalar.copy(out=xpT[:, k * 128 : (k + 1) * 128], in_=pt[:])

            # ---- layer-0 front: x' = dis * x ----
            # (big-buffer tag sharing to fit SBUF: xnm->gT, xp->uw, outb->gnm)
            x_nm = bigp.tile([128, KB * F], f32, tag="gT")
            nc.sync.dma_start(out=x_nm[:], in_=x_in[:, :])
            xp0 = bigp.tile([128, KB * F], f32, tag="uw")
            for k in range(KB):
                nc.scalar.mul(
                    out=xp0[:, k * F : (k + 1) * F],
                    in_=x_nm[:, k * F : (k + 1) * F],
                    mul=dis_sb[:, k : k + 1],
                )
            transpose_to_xpT(xp0)

            for _rep in range(reps):
                for l in range(nlayers):
                    # ---- A: g^T = W^T @ x'^T ----
                    gT = bigp.tile([F, SH], f32, tag="gT")
                    CH = 512
                    for c0 in range(0, SH, CH):
                        cw = min(CH, SH - c0)
                        pm = ps_mm.tile([F, CH], f32, space="PSUM", tag="mm")
                        nc.tensor.matmul(
                            out=pm[:, :cw],
                            lhsT=w_sb[l][:],
                            rhs=xpT[:, c0 : c0 + cw],
                            start=True,
                            stop=True,
                        )
                        nc.scalar.copy(out=gT[:, c0 : c0 + cw], in_=pm[:, :cw])

                    # ---- B: transpose to node-major + store + AllGather ----
                    g_nm = bigp.tile([128, KB * F], f32, tag="gnm")
                    for k in range(KB):
                        pt = ps_tr.tile([128, F], f32, space="PSUM", tag="trB")
                        nc.tensor.transpose(
                            out=pt[:],
                            in_=gT[:, k * 128 : (k + 1) * 128],
                            identity=ident[:64, :64],
                        )
                        nc.scalar.copy(out=g_nm[:, k * F : (k + 1) * F], in_=pt[:])
                    # halves: partitions 0..63 -> part1 rows, 64..127 -> part2 rows
                    nc.sync.dma_start(out=g_shard[0:HALF, :], in_=g_nm[0:64, :])
                    nc.sync.dma_start(out=g_shard[HALF:SH, :], in_=g_nm[64:128, :])
                    if not skip_ag:
                        nc.gpsimd.collective_compute(
                            "AllGather",
                            mybir.AluOpType.bypass,
                            replica_groups=[list(range(NCORES))],
                            ins=[g_shard[0:HALF, :].opt()],
                            outs=[table[0:P1_SIZE, :].opt()],
                        )
                        nc.gpsimd.collective_compute(
                            "AllGather",
                            mybir.AluOpType.bypass,
                            replica_groups=[list(range(NCORES))],
                            ins=[g_shard[HALF:SH, :].opt()],
                            outs=[table[P2_OFF : P2_OFF + P1_SIZE, :].opt()],
                        )

                    if stop == "ag":
                        dbg = sb.tile([128, F], f32, tag="dbg")
                        nc.sync.dma_start(out=dbg[:], in_=table[0:128, :])
                        nc.sync.dma_start(out=x_out[l][:, 0:F], in_=dbg[:])
                        continue

                    # ---- C: window gathers + segment reduces ----
                    for w in range(nwin):
                        u_w = bigp.tile([128, KB * F], f32, tag="uw")
                        in_ap = (
                            table[BASE_1:P2_OFF, :]
                            if w == 0
                            else table[BASE_2:, :]
                        )
                        off8 = 0
                        for g in groups[w][:max_groups]:
                            gcols = sum(int(D[w][kb]) for kb in g) + 1
                            nidx = gcols * 128
                            q = next_q()
                            idx_sb = sb.tile([128, nidx // 16], i16, tag=f"idx{q}")
                            nc.sync.dma_start(
                                out=idx_sb[:],
                                in_=idx_in[w][:, off8 : off8 + nidx // 16],
                            )
                            msgs = msp.tile(
                                [128, meta["maxg"], F], f32, tag=f"msgs{q}"
                            )
                            nc.gpsimd.dma_gather(
                                out_ap=msgs[:, :gcols, :],
                                in_ap=in_ap,
                                idxs_ap=idx_sb[:],
                                num_idxs=nidx,
                                num_idxs_reg=nidx,
                                elem_size=F,
                                single_packet=False,
                                queue_num=q,
                            )
                            if stop == "gather":
                                nc.scalar.copy(
                                    out=u_w[:, 0:F],
                                    in_=msgs[:, 0, :],
                                )
                            else:
                                loc = 0
                                for kb in g:
                                    d = int(D[w][kb])
                                    nc.vector.tensor_reduce(
                                        out=u_w[:, kb * F : (kb + 1) * F],
                                        in_=msgs[:, loc : loc + d, :].rearrange(
                                            "p d f -> p f d"
                                        ),
                                        axis=mybir.AxisListType.X,
                                        op=mybir.AluOpType.add,
                                    )
                                    loc += d
                            off8 += nidx // 16
                        nc.sync.dma_start(out=u_dram[w][:, :], in_=u_w[:])
                    if stop in ("gather", "reduce"):
                        dbg = sb.tile([128, F], f32, tag="dbg")
                        nc.sync.dma_start(out=dbg[:], in_=u_dram[0][0:128, :])
                        nc.sync.dma_start(out=x_out[l][:, 0:F], in_=dbg[:])
                        continue

                    # ---- D: align + add ----
                    u_al = []
                    for w in range(2):
                        t = bigp.tile([128, KB, F], f32, tag="gT" if w else "ual0")
                        nc.gpsimd.dma_gather(
                            out_ap=t[:],
                            in_ap=u_dram[w][:, :],
                            idxs_ap=al_sb[w][:],
                            num_idxs=SH,
                            num_idxs_reg=SH,
                            elem_size=F,
                            single_packet=False,
                            queue_num=next_q(),
                        )
                        u_al.append(t)
                    u = u_al[0][:].rearrange("p k f -> p (k f)")
                    nc.vector.tensor_tensor(
                        out=u,
                        in0=u,
                        in1=u_al[1][:].rearrange("p k f -> p (k f)"),
                        op=mybir.AluOpType.add,
                    )
                    if stop == "align":
                        nc.sync.dma_start(out=x_out[l][:, :], in_=u)
                        continue

                    # ---- E: postproc ----
                    out_sb = bigp.tile([128, KB * F], f32, tag="gnm")
                    if l < 2:
                        xp = bigp.tile([128, KB * F], f32, tag="uw")
                    if not with_bias:
                        # r = relu(u); out = dis*r; x' = dis2*r
                        r = u_al[1][:].rearrange("p k f -> p (k f)")
                        if l < 2:
                            nc.scalar.activation(
                                out=r, in_=u, func=mybir.ActivationFunctionType.Relu
                            )
                        for k in range(KB):
                            s = slice(k * F, (k + 1) * F)
                            if l < 2:
                                nc.scalar.mul(
                                    out=out_sb[:, s], in_=r[:, s], mul=dis_sb[:, k : k + 1]
                                )
                                nc.scalar.mul(
                                    out=xp[:, s], in_=r[:, s], mul=dis2_sb[:, k : k + 1]
                                )
                            else:
                                nc.scalar.mul(
                                    out=out_sb[:, s], in_=u[:, s], mul=dis_sb[:, k : k + 1]
                                )
                    else:
                        # v = dis*u ; t = relu(v + b) (layers 1,2) / t = v + b (layer 3)
                        # out = t ; x' = dis*t
                        v = u_al[1][:].rearrange("p k f -> p (k f)")
                        for k in range(KB):
                            s = slice(k * F, (k + 1) * F)
                            nc.scalar.mul(
                                out=v[:, s], in_=u[:, s], mul=dis_sb[:, k : k + 1]
                            )
                        bb = b_sb[l][:].to_broadcast([128, F])
                        for k in range(KB):
                            s = slice(k * F, (k + 1) * F)
                            nc.vector.tensor_tensor(
                                out=out_sb[:, s], in0=v[:, s], in1=bb, op=mybir.AluOpType.add
                            )
                        if l < 2:
                            nc.scalar.activation(
                                out=out_sb[:],
                                in_=out_sb[:],
                                func=mybir.ActivationFunctionType.Relu,
                            )
                            for k in range(KB):
                                s = slice(k * F, (k + 1) * F)
                                nc.scalar.mul(
                                    out=xp[:, s],
                                    in_=out_sb[:, s],
                                    mul=dis_sb[:, k : k + 1],
                                )
                    nc.sync.dma_start(out=x_out[l][:, :], in_=out_sb[:])

                    # ---- F: next-layer x'^T ----
                    if l < 2:
                        transpose_to_xpT(xp)

    nc.compile()
    return nc


# ============================ entry point =================================

def _get_compiled(edge_index, biases_zero, reps):
    key = ("k", int(np.asarray(edge_index).sum() & 0xFFFFFFF), biases_zero, reps)
    if key not in _CACHE:
        meta = _preprocess(np.asarray(edge_index))
        nc = _build(meta, with_bias=not biases_zero, reps=reps)
        _CACHE[key] = (meta, nc)
    return _CACHE[key]


def _prepare(x, edge_index, W1, b1, W2, b2, W3, b3, _reps=1):
    x = np.asarray(x, np.float32)
    biases_zero = all(
        float(np.abs(np.asarray(b)).max()) == 0.0 for b in (b1, b2, b3)
    )
    meta, nc = _get_compiled(edge_index, biases_zero, _reps)
    percore, deg_arr = meta["percore"], meta["deg_arr"]

    W3p = np.zeros((F, F), np.float32)
    W3p[:, :OC] = np.asarray(W3, np.float32)
    b3p = np.zeros((F,), np.float32)
    b3p[:OC] = np.asarray(b3, np.float32)
    Ws = [np.asarray(W1, np.float32), np.asarray(W2, np.float32), W3p]
    bs = [
        np.asarray(b1, np.float32).reshape(1, F),
        np.asarray(b2, np.float32).reshape(1, F),
        b3p.reshape(1, F),
    ]

    in_maps = []
    for c in range(NCORES):
        nodes_c = percore[c]
        x_nm = np.zeros((128, KB * F), np.float32)
        jj = np.arange(len(nodes_c))
        kk, pp = jj // 128, jj % 128
        x_nm_3d = x_nm.reshape(128, KB, F)
        x_nm_3d[pp, kk, :] = x[nodes_c]
        m = {
            "x_nm": x_nm,
            "deg": deg_arr[c],
            "w1": Ws[0], "w2": Ws[1], "w3": Ws[2],
            "b1": bs[0], "b2": bs[1], "b3": bs[2],
            "idxw1": meta["idx_blob"][0][c],
            "idxw2": meta["idx_blob"][1][c],
            "alw1": meta["al_idx"][0][c],
            "alw2": meta["al_idx"][1][c],
        }
        in_maps.append(m)

    return meta, nc, in_maps


def kernel(x, edge_index, W1, b1, W2, b2, W3, b3, _reps=1):
    meta, nc, in_maps = _prepare(x, edge_index, W1, b1, W2, b2, W3, b3, _reps)
    percore = meta["percore"]
    res = bass_utils.run_bass_kernel_spmd(nc, in_maps, core_ids=list(range(NCORES)))

    # unshard: x_out tiles [128, KB*F] -> per-node rows
    out = np.empty((N, 160), np.float32)
    for c in range(NCORES):
        nodes_c = percore[c]
        jj = np.arange(len(nodes_c))
        kk, pp = jj // 128, jj % 128
        x1 = res.results[c]["x1o"].reshape(128, KB, F)[pp, kk, :]
        x2 = res.results[c]["x2o"].reshape(128, KB, F)[pp, kk, :]
        x3 = res.results[c]["x3o"].reshape(128, KB, F)[pp, kk, :OC]
        out[nodes_c] = np.concatenate([x3, x2, x1], axis=1)
    return out

